# revision 33
# baseline (speedup 1.0000x reference)
"""Expert-parallel SwiGLU MLP (MoE experts) for 8 Trainium2 NeuronCores.

Problem: routed_in_egD [E*G, D] fp32, w1/w3 [E, D, F], w2 [E, F, D], E=8,
G=2048, D=2048, F=5632.  reference:
    x_egD = routed.reshape(E, G, D)
    mid   = silu(x @ w1) * (x @ w3)          # [E, G, F]
    out   = (mid @ w2).reshape(E*G, D)

Sharding: expert-parallel - core e gets expert e's x slice + weights; no
collectives.  Each core runs three 2048x2048x5632-class GEMMs (~142 GFLOP),
matmul-roofline-bound at the bf16 PE rate (1 col/cycle @ 2.4 GHz -> ~1.80 ms
of pure matmul streaming per core), so the whole game is keeping the PE at
~100% MATMUL occupancy: no phase-boundary stalls, minimal ramp.

Per-core kernel (all matmuls bf16, PSUM fp32):
  phase A: x g-blocks 0/1 (the ramp-critical half) are transposed on the PE
           (fp32 transpose + DVE copy-cast into xTb, v1-proven path - the
           xbar DMA-transpose is unpredictable under queue concurrency);
           blocks 2/3 are staged via SWDGE DRAM->DRAM bf16 cast + xbar
           DMA-transpose, interleaved into the gh=0 weight stream (needed
           only by gh=1, ~700us later).  xTb[b] [P, DO, 512], d = do*128+p.
  phase 1: gh-outer (g-halves): per (gh, fc): gateT/upT = w1/w3.T @ x
           accumulated over d in PSUM; SwiGLU (ACT silu -> bf16, DVE mul);
           midT spilled to DRAM bf16 per-gp tiles => precise DMA deps.
           w1/w3 re-read per gh (DMA has headroom; the PE does not).  The
           dq=0 w2 panel is cast-loaded into a pre-allocated buffer during
           gh=0, and mq panels prefetch on the idle sync queue during the
           phase-1 tail, so phase 2 starts with zero DMA wait.
  phase 2: out[g,d] = sum_f midT[f,g]*w2[f,d]: mid panels stationary (bf16),
           w2 panels DMA-cast fp32->bf16 (moving), PSUM accumulation over F.
"""

import numpy as np

import concourse.mybir as mybir
import concourse.tile as tile
from concourse import bacc
from concourse.bass_utils import run_bass_kernel_spmd
from concourse.masks import make_identity

E, G, D, F = 8, 2048, 2048, 5632
P = 128
DO = D // P      # 16 d-chunks (contraction steps)
FC = F // P      # 44 f-chunks
GB = 4           # g-blocks of 512 for the x transpose

F32 = mybir.dt.float32
BF16 = mybir.dt.bfloat16


def build_nc():
    nc = bacc.Bacc("TRN2", target_bir_lowering=False)
    x = nc.dram_tensor("x", [G, D], F32, kind="ExternalInput").ap()
    w1 = nc.dram_tensor("w1", [D, F], F32, kind="ExternalInput").ap()
    w2 = nc.dram_tensor("w2", [F, D], F32, kind="ExternalInput").ap()
    w3 = nc.dram_tensor("w3", [D, F], F32, kind="ExternalInput").ap()
    out = nc.dram_tensor("out", [G, D], F32, kind="ExternalOutput").ap()

    # both transpose paths map rows as d = do*128 + p (natural d-chunking)
    w1r = w1.rearrange("(do p) f -> p do f", p=P)
    w3r = w3.rearrange("(do p) f -> p do f", p=P)
    w2r = w2.rearrange("(fo p) d -> p fo d", p=P)

    with tile.TileContext(nc) as tc:
        dram = tc.alloc_tile_pool(name="dram", bufs=1, space="DRAM")
        # per-gp mid tiles: mids[gp][p, fo, g'] = mid for f = fo*128+p,
        # g = gp*256 + g'.  Phase-1 writes are per-partition contiguous 512B;
        # phase-2 panel reads are per-partition contiguous 22KB.
        mids = [
            dram.tile([P, FC, 256], BF16, tag=f"mid{gp}", name=f"mid{gp}")
            for gp in range(8)
        ]

        # right-side pools pre-allocated so phase-2 inputs stream in during
        # phase 1 (their SBUF space never overlaps the phase-1 pools)
        mqp = tc.alloc_tile_pool(name="mqp", bufs=2, side="right")
        w2p0 = tc.alloc_tile_pool(name="w2p0", bufs=1, side="right")
        w2q0 = w2p0.tile([P, FC, 512], BF16, name="w2q0")

        xtp = tc.alloc_tile_pool(name="xtp", bufs=1)
        xTb = [
            xtp.tile([P, DO, 512], BF16, tag=f"xTb{b}", name=f"xTb{b}")
            for b in range(GB)
        ]

        wp = tc.alloc_tile_pool(name="wp", bufs=4)
        mp = tc.alloc_tile_pool(name="mp", bufs=4)

        def load_w(fc):
            w1t = wp.tile([P, DO, P], BF16, tag="w1", name=f"w1t{fc}")
            nc.gpsimd.dma_start(w1t, w1r[:, :, fc * P : (fc + 1) * P])
            w3t = wp.tile([P, DO, P], BF16, tag="w3", name=f"w3t{fc}")
            nc.gpsimd.dma_start(w3t, w3r[:, :, fc * P : (fc + 1) * P])
            return w1t, w3t

        # ---- phase A: weight loads for fc0/fc1 first on the SWDGE FIFO,
        # then PE-transpose ALL x blocks while those loads stream.  The xbar
        # DMA-transpose alternative is off the PE but measures 45-130 GB/s
        # under queue concurrency and its completion sems alias with the
        # phase-1 mid-write DMAs (30-50us PE stalls); the PE path costs a
        # predictable ~27us of transpose streaming, hidden under the 16MB
        # x load (~46us on sync).
        head_w = [load_w(0), load_w(1)]
        idp = tc.alloc_tile_pool(name="idp", bufs=1)
        p0 = tc.alloc_tile_pool(name="p0", bufs=6)
        p0ps = tc.alloc_tile_pool(name="p0ps", bufs=4, space="PSUM")
        ident = idp.tile([P, P], F32, name="ident")
        make_identity(nc, ident)
        for b in range(GB):
            for gi in range(4):
                g0 = (b * 4 + gi) * P
                for q in range(4):
                    xsq = p0.tile([P, 512], F32, tag="xs", name="xsq")
                    nc.sync.dma_start(xsq, x[g0 : g0 + P, q * 512 : (q + 1) * 512])
                    tp = p0ps.tile([P, 4, P], F32, tag="tp", name="tp")
                    for j in range(4):
                        nc.tensor.transpose(
                            tp[:, j], xsq[:, j * P : (j + 1) * P], ident
                        )
                    nc.vector.tensor_copy(
                        xTb[b][:, q * 4 : (q + 1) * 4, gi * P : (gi + 1) * P], tp
                    )
        p0ps.release()
        p0.release()
        idp.release()

        # ---- phase 1: midT[f, g] = silu(w1.T x) * (w3.T x), spill bf16
        ps1g = tc.alloc_tile_pool(name="ps1g", bufs=2, space="PSUM")
        ps1u = tc.alloc_tile_pool(name="ps1u", bufs=2, space="PSUM")
        w2bounds = [0, 6, 12, 18, 24, 29, 34, 39, 44]

        def p1_mms(pg, pu, w1t, w3t, xs, j):
            for d in range(DO):
                st, sp_ = (d == 0), (d == DO - 1)
                nc.tensor.matmul(pg[:, j], w1t[:, d], xs[:, d], start=st, stop=sp_)
                nc.tensor.matmul(pu[:, j], w3t[:, d], xs[:, d], start=st, stop=sp_)

        def p1_finish(gh, fc, pg, pu):
            mo = mp.tile([P, 4, 256], BF16, tag="mo", name="mo")
            nc.scalar.activation(
                mo, pg.rearrange("p j g -> p (j g)"),
                mybir.ActivationFunctionType.Silu,
            )
            nc.vector.tensor_mul(mo, mo, pu.rearrange("p j g -> p (j g)"))
            for k in range(4):
                nc.scalar.dma_start(mids[gh * 4 + k][:, fc], mo[:, k])

        for gh in range(2):
            for fc in range(FC):
                if gh == 0 and fc < 2:
                    w1t, w3t = head_w[fc]
                else:
                    w1t, w3t = load_w(fc)
                if gh == 0 and 6 <= fc < 14:
                    # stream the dq=0 w2 panel into its pre-allocated buffer
                    # while the SWDGE queue has slack
                    lo, hi = w2bounds[fc - 6], w2bounds[fc - 5]
                    nc.gpsimd.dma_start(w2q0[:, lo:hi, :], w2r[:, lo:hi, 0:512])
                pg = ps1g.tile([P, 2, 512], F32, tag="pg")
                pu = ps1u.tile([P, 2, 512], F32, tag="pu")
                for j in range(2):
                    p1_mms(pg, pu, w1t, w3t, xTb[gh * 2 + j], j)
                p1_finish(gh, fc, pg, pu)
        mp.release()
        wp.release()
        xtp.release()
        ps1u.release()
        ps1g.release()

        # ---- phase 2: out[g, d] = midT.T @ w2 (bf16 x bf16, fp32 psum)
        w2p = tc.alloc_tile_pool(name="w2p", bufs=2, side="right")
        op = tc.alloc_tile_pool(name="op", bufs=4, side="right")
        ps2 = tc.alloc_tile_pool(name="ps2", bufs=3, space="PSUM")
        for dq in range(4):
            if dq == 0:
                w2q = w2q0
            else:
                w2q = w2p.tile([P, FC, 512], BF16, tag="w2q")
                nc.gpsimd.dma_start(w2q, w2r[:, :, dq * 512 : (dq + 1) * 512])
            for gp in range(8):
                mq = mqp.tile([P, FC, 256], BF16, tag="mq")
                # all mq panels on the scalar queue (idle in phase 2): one
                # stream per HWDGE ring instead of interleaving with the
                # out-stores on sync
                nc.scalar.dma_start(mq, mids[gp])
                po = ps2.tile([P, 2, 512], F32, tag="po")
                for fo in range(FC):
                    st, sp_ = (fo == 0), (fo == FC - 1)
                    for gc in range(2):
                        nc.tensor.matmul(
                            po[:, gc],
                            mq[:, fo, gc * P : (gc + 1) * P],
                            w2q[:, fo],
                            start=st,
                            stop=sp_,
                        )
                for gc in range(2):
                    ot = op.tile([P, 512], F32, tag="ot")
                    nc.vector.tensor_copy(ot, po[:, gc])
                    g00 = (gp * 2 + gc) * P
                    nc.sync.dma_start(
                        out[g00 : g00 + P, dq * 512 : (dq + 1) * 512], ot
                    )
        op.release()
        w2p.release()
        ps2.release()
        w2p0.release()
        mqp.release()
        dram.release()
    nc.compile()
    return nc


_NC_CACHE = None


def _get_nc():
    global _NC_CACHE
    if _NC_CACHE is None:
        _NC_CACHE = build_nc()
    return _NC_CACHE


def _in_maps(routed_in_egD, w1, w2, w3):
    x = np.ascontiguousarray(np.asarray(routed_in_egD, dtype=np.float32))
    w1 = np.ascontiguousarray(np.asarray(w1, dtype=np.float32))
    w2 = np.ascontiguousarray(np.asarray(w2, dtype=np.float32))
    w3 = np.ascontiguousarray(np.asarray(w3, dtype=np.float32))
    x_e = x.reshape(E, G, D)
    return [
        {"x": x_e[e], "w1": w1[e], "w2": w2[e], "w3": w3[e]} for e in range(E)
    ]


def kernel(routed_in_egD, w1, w2, w3):
    nc = _get_nc()
    in_maps = _in_maps(routed_in_egD, w1, w2, w3)
    try:
        res = run_bass_kernel_spmd(nc, in_maps, core_ids=list(range(E)))
    except Exception:
        # the first execute after process start occasionally dies with a
        # transient NRT_EXEC_UNIT_UNRECOVERABLE through the PJRT tunnel;
        # a straight retry has always succeeded
        res = run_bass_kernel_spmd(nc, in_maps, core_ids=list(range(E)))
    return np.concatenate([r["out"] for r in res.results], axis=0)


def run_traced(routed_in_egD, w1, w2, w3, **trace_kwargs):
    """For test.py: run with NTFF tracing; returns (full_out, BassKernelResults)."""
    nc = _get_nc()
    res = run_bass_kernel_spmd(
        nc,
        _in_maps(routed_in_egD, w1, w2, w3),
        core_ids=list(range(E)),
        trace=True,
        **trace_kwargs,
    )
    out = np.concatenate([r["out"] for r in res.results], axis=0)
    return out, res
